# revision 16
# baseline (speedup 1.0000x reference)
"""GPTQ 4-bit dequant + linear (x @ W.T + bias) on 8 Trainium2 NeuronCores.

Problem shapes (hardcoded):
  x       [4, 2048, 4096] f32   -> host-cast to bf16 and pre-transposed
  qweight [16384, 512]    i32   (8x 4-bit nibbles per int32 along K)
  qzeros  [16384, 4]      i32
  scales  [16384, 32]     f32
  bias    [16384]         f32
  out     [4, 2048, 16384] f32

Sharding: column-parallel over out_features. Each of the 8 cores gets a
2048-row slab of qweight/qzeros/scales/bias, x replicated; outputs are
concatenated on the host along the feature axis.

Per-core kernel:
  Phase A: dequantize the int4 slab to bf16 W [n, k] on DVE (nibble extract +
           per-group (q-z)*s with per-partition scalars), then transpose
           128x128 tiles on the TENSOR engine (idle during this phase) into a
           persistent SBUF-resident W.T [128 k-low, 32 k-chunk, 2048 n],
           batching 4 tiles per PSUM scratch bank, copied out on ACT.
  Phase B: per 128-token chunk: ONE contiguous DMA of pre-transposed bf16 x
           (8 KiB per partition), 4x32 PE matmuls accumulating [128t, 512n]
           PSUM groups over K in two 2-bank pair tiles, PSUM->SBUF drain
           fused with bias add on DVE once per pair (each pair drains while
           the other pair's matmuls run), DMA out.
"""
import sys

for _p in ("/opt/trn_rl_repo", "/root/.axon_site/_ro/trn_rl_repo"):
    if _p not in sys.path:
        sys.path.append(_p)

import numpy as np
import ml_dtypes
import concourse.bass as bass
import concourse.mybir as mybir
from concourse import tile, bacc
from concourse.bass_utils import run_bass_kernel_spmd
from concourse.masks import make_identity

BF16 = mybir.dt.bfloat16
F32 = mybir.dt.float32
I32 = mybir.dt.int32

B, S, K, N = 4, 2048, 4096, 16384
T = B * S                      # 8192 tokens
NCORES = 8
NS = N // NCORES               # 2048 out features per core
PACK = 8
GS = 128                       # quant group size
G = K // GS                    # 32 groups
TCH = 128                      # tokens per chunk
KC = K // 128                  # 32 k-chunks
MMN = 512                      # matmul moving free dim (one PSUM bank of f32)
NBLK = NS // MMN               # 4

_LSR = mybir.AluOpType.logical_shift_right
_AND = mybir.AluOpType.bitwise_and
_SUB = mybir.AluOpType.subtract
_MUL = mybir.AluOpType.mult
_ADD = mybir.AluOpType.add
_BYP = mybir.AluOpType.bypass


def build(t_total: int = T):
    nt = t_total // TCH
    nc = bacc.Bacc("TRN2", target_bir_lowering=False, debug=False)
    xt_d = nc.dram_tensor("xt", [nt, TCH, K], BF16, kind="ExternalInput")
    qw_d = nc.dram_tensor("qw", [NS, K // PACK], I32, kind="ExternalInput")
    qz_d = nc.dram_tensor("qz", [NS, G // PACK], I32, kind="ExternalInput")
    sc_d = nc.dram_tensor("sc", [NS, G], F32, kind="ExternalInput")
    b_d = nc.dram_tensor("b", [NS], F32, kind="ExternalInput")
    out_d = nc.dram_tensor("out", [t_total, NS], F32, kind="ExternalOutput")

    with tile.TileContext(nc) as tc:
        with (
            tc.tile_pool(name="wtp", bufs=1) as wtpool,
            tc.tile_pool(name="consts", bufs=1) as cpool,
            tc.tile_pool(name="aload", bufs=2) as apool,
            tc.tile_pool(name="anib", bufs=1) as nibpool,
            tc.tile_pool(name="awch", bufs=2) as wchpool,
            tc.tile_pool(name="bx", bufs=2) as bxpool,
            tc.tile_pool(name="bout", bufs=2) as bopool,
            tc.tile_pool(name="ps", bufs=1, space=bass.MemorySpace.PSUM) as pspool,
            tc.tile_pool(name="ptr", bufs=2, space=bass.MemorySpace.PSUM) as trpool,
            tc.tile_pool(name="pwm", bufs=2, space=bass.MemorySpace.PSUM) as wmpool,
        ):
            # persistent dequantized W.T: [128 k-low, KC k-chunk, NS n] bf16
            wT = wtpool.tile([128, KC, NS], BF16)

            ident = cpool.tile([128, 128], BF16)
            make_identity(nc, ident[:])
            ones_t = cpool.tile([1, 128], BF16)
            nc.vector.memset(ones_t[:], 1.0)
            bias_row = cpool.tile([1, NS], BF16)
            nc.gpsimd.dma_start(bias_row[:], b_d[:].rearrange("(o n) -> o n", o=1))

            # bias broadcast to all 128 partitions via outer-product matmuls
            bias_full = cpool.tile([128, NS], BF16)
            for p in range(2):
                bias_ps = pspool.tile([128, 2 * MMN], F32, name=f"ps{'AB'[p]}")
                for h in range(2):
                    nb = p * 2 + h
                    nc.tensor.matmul(
                        bias_ps[:, h * MMN:(h + 1) * MMN], ones_t[:],
                        bias_row[:, nb * MMN:(nb + 1) * MMN],
                        start=True, stop=True)
                nc.scalar.copy(bias_full[:, p * 2 * MMN:(p + 1) * 2 * MMN],
                               bias_ps[:])

            # ---- Phase A: dequantize weight slab, n-chunks of 128 rows
            for j in range(NS // 128):
                n0 = j * 128
                qw_t = apool.tile([128, K // PACK], I32)
                nc.sync.dma_start(qw_t[:], qw_d[n0:n0 + 128, :])
                qz_t = apool.tile([128, G // PACK], I32)
                nc.sync.dma_start(qz_t[:], qz_d[n0:n0 + 128, :])
                sc_t = apool.tile([128, G], F32)
                nc.sync.dma_start(sc_t[:], sc_d[n0:n0 + 128, :])

                zi_t = apool.tile([128, G], I32)
                for i in range(PACK):
                    nc.vector.tensor_scalar(
                        out=zi_t[:, i::PACK], in0=qz_t[:],
                        scalar1=4 * i, scalar2=0xF, op0=_LSR, op1=_AND)
                z_t = apool.tile([128, G], F32)
                nc.vector.tensor_copy(z_t[:], zi_t[:])
                # neg_zs = -z*s, the per-group bias for ACT-side dequant
                neg_zs = apool.tile([128, G], F32)
                nc.vector.scalar_tensor_tensor(
                    out=neg_zs[:], in0=z_t[:], scalar=-1.0,
                    in1=sc_t[:], op0=_MUL, op1=_MUL)

                w_t = wchpool.tile([128, K], BF16)
                for h in range(2):
                    nib_t = nibpool.tile([128, K // 2], I32)
                    for i in range(PACK):
                        nc.vector.tensor_scalar(
                            out=nib_t[:, i::PACK],
                            in0=qw_t[:, h * (K // PACK // 2):(h + 1) * (K // PACK // 2)],
                            scalar1=4 * i, scalar2=0xF, op0=_LSR, op1=_AND)
                    # per-group (q-z)*s: split between DVE (tensor_scalar) and
                    # ACT (activation: Identity(nib*s + (-z*s)))
                    for g16 in range(G // 2):
                        g = h * (G // 2) + g16
                        if g16 < 8:
                            nc.vector.tensor_scalar(
                                out=w_t[:, g * GS:(g + 1) * GS],
                                in0=nib_t[:, g16 * GS:(g16 + 1) * GS],
                                scalar1=z_t[:, g:g + 1], scalar2=sc_t[:, g:g + 1],
                                op0=_SUB, op1=_MUL)
                        else:
                            nc.scalar.activation(
                                out=w_t[:, g * GS:(g + 1) * GS],
                                in_=nib_t[:, g16 * GS:(g16 + 1) * GS],
                                func=mybir.ActivationFunctionType.Identity,
                                bias=neg_zs[:, g:g + 1], scale=sc_t[:, g:g + 1])

                # transpose 128x128 tiles on PE (otherwise idle), 8 per PSUM
                # scratch bank, batched ACT copy into the resident wT
                for q in range(KC // 8):
                    tr_t = trpool.tile([128, 1024], BF16)
                    for r in range(8):
                        c = q * 8 + r
                        nc.tensor.transpose(
                            tr_t[:, r * 128:(r + 1) * 128],
                            w_t[:, c * 128:(c + 1) * 128], ident[:])
                    src = tr_t[:].rearrange("p (c n) -> p c n", c=8)
                    nc.scalar.copy(wT[:, q * 8:(q + 1) * 8, n0:n0 + 128], src)
                    # keep-warm: dummy transposes into a never-read scratch
                    # bank fill the PE's supply gaps during phase A so the HAM
                    # activity monitor holds the 2.4 GHz clock
                    warm_t = wmpool.tile([128, 512], BF16)
                    for r in range(4):
                        nc.tensor.transpose(
                            warm_t[:, r * 128:(r + 1) * 128],
                            w_t[:, (q * 8 + 2 * r) * 128:(q * 8 + 2 * r + 1) * 128],
                            ident[:])

            # ---- Phase B: stream tokens
            for ti in range(nt):
                xT_t = bxpool.tile([128, K], BF16)
                nc.sync.dma_start(xT_t[:], xt_d[ti])
                o_t = bopool.tile([128, NS], F32)
                for p in range(2):
                    ps = pspool.tile([128, 2 * MMN], F32, name=f"ps{'AB'[p]}")
                    for h in range(2):
                        nb = p * 2 + h
                        sl = slice(nb * MMN, (nb + 1) * MMN)
                        for c in range(KC):
                            nc.tensor.matmul(
                                ps[:, h * MMN:(h + 1) * MMN],
                                xT_t[:, c * 128:(c + 1) * 128],
                                wT[:, c, sl],
                                start=(c == 0), stop=(c == KC - 1))
                    psl = slice(p * 2 * MMN, (p + 1) * 2 * MMN)
                    nc.vector.scalar_tensor_tensor(
                        out=o_t[:, psl], in0=ps[:], scalar=0.0,
                        in1=bias_full[:, psl], op0=_BYP, op1=_ADD)
                nc.scalar.dma_start(out_d[ti * TCH:(ti + 1) * TCH, :], o_t[:])

    nc.compile()
    return nc


_nc_cache = {}


def _get_nc(t_total: int = T):
    if t_total not in _nc_cache:
        _nc_cache[t_total] = build(t_total)
    return _nc_cache[t_total]


def _prep_x(x, t_total: int = T):
    """f32 [B,S,K] -> bf16 [nt, 128 k-low, (KC k-chunk, 128 tok)] pre-transposed."""
    nt = t_total // TCH
    xb = np.asarray(x).reshape(t_total, K).astype(ml_dtypes.bfloat16)
    # [ti, t, c, kl] -> [ti, kl, c, t]
    xr = xb.reshape(nt, TCH, KC, 128).transpose(0, 3, 2, 1)
    return np.ascontiguousarray(xr).reshape(nt, 128, K)


def kernel(x, qweight, qzeros, scales, bias, trace=False):
    xt = _prep_x(x)
    in_maps = []
    for c in range(NCORES):
        sl = slice(c * NS, (c + 1) * NS)
        in_maps.append({
            "xt": xt,
            "qw": np.ascontiguousarray(qweight[sl]),
            "qz": np.ascontiguousarray(qzeros[sl]),
            "sc": np.ascontiguousarray(scales[sl]),
            "b": np.ascontiguousarray(bias[sl]),
        })
    nc = _get_nc()
    res = run_bass_kernel_spmd(nc, in_maps, core_ids=list(range(NCORES)),
                               trace=trace)
    out = np.concatenate([r["out"] for r in res.results], axis=1)
    out = out.reshape(B, S, N).astype(np.float32, copy=False)
    if trace:
        return out, res
    return out


# revision 19
# speedup vs baseline: 1.0012x; 1.0012x over previous
"""GPTQ 4-bit dequant + linear (x @ W.T + bias) on 8 Trainium2 NeuronCores.

Problem shapes (hardcoded):
  x       [4, 2048, 4096] f32   -> host-cast to bf16 and pre-transposed
  qweight [16384, 512]    i32   (8x 4-bit nibbles per int32 along K)
  qzeros  [16384, 4]      i32
  scales  [16384, 32]     f32
  bias    [16384]         f32
  out     [4, 2048, 16384] f32

Sharding: column-parallel over out_features. Each of the 8 cores gets a
2048-row slab of qweight/qzeros/scales/bias, x replicated; outputs are
concatenated on the host along the feature axis.

Per-core kernel:
  Phase A: dequantize the int4 slab to bf16 W [n, k] on DVE (nibble extract +
           per-group (q-z)*s with per-partition scalars), then transpose
           128x128 tiles on the TENSOR engine (idle during this phase) into a
           persistent SBUF-resident W.T [128 k-low, 32 k-chunk, 2048 n],
           batching 4 tiles per PSUM scratch bank, copied out on ACT.
  Phase B: per 128-token chunk: ONE contiguous DMA of pre-transposed bf16 x
           (8 KiB per partition), 4x32 PE matmuls accumulating [128t, 512n]
           PSUM groups over K in two 2-bank pair tiles, PSUM->SBUF drain
           fused with bias add on DVE once per pair (each pair drains while
           the other pair's matmuls run), DMA out.
"""
import sys

for _p in ("/opt/trn_rl_repo", "/root/.axon_site/_ro/trn_rl_repo"):
    if _p not in sys.path:
        sys.path.append(_p)

import numpy as np
import ml_dtypes
import concourse.bass as bass
import concourse.mybir as mybir
from concourse import tile, bacc
from concourse.bass_utils import run_bass_kernel_spmd
from concourse.masks import make_identity

BF16 = mybir.dt.bfloat16
F32 = mybir.dt.float32
I32 = mybir.dt.int32

B, S, K, N = 4, 2048, 4096, 16384
T = B * S                      # 8192 tokens
NCORES = 8
NS = N // NCORES               # 2048 out features per core
PACK = 8
GS = 128                       # quant group size
G = K // GS                    # 32 groups
TCH = 128                      # tokens per chunk
KC = K // 128                  # 32 k-chunks
MMN = 512                      # matmul moving free dim (one PSUM bank of f32)
NBLK = NS // MMN               # 4

_LSR = mybir.AluOpType.logical_shift_right
_AND = mybir.AluOpType.bitwise_and
_SUB = mybir.AluOpType.subtract
_MUL = mybir.AluOpType.mult
_ADD = mybir.AluOpType.add
_BYP = mybir.AluOpType.bypass


def build(t_total: int = T):
    nt = t_total // TCH
    nc = bacc.Bacc("TRN2", target_bir_lowering=False, debug=False)
    xt_d = nc.dram_tensor("xt", [nt, TCH, K], BF16, kind="ExternalInput")
    qw_d = nc.dram_tensor("qw", [NS, K // PACK], I32, kind="ExternalInput")
    qz_d = nc.dram_tensor("qz", [NS, G // PACK], I32, kind="ExternalInput")
    sc_d = nc.dram_tensor("sc", [NS, G], F32, kind="ExternalInput")
    b_d = nc.dram_tensor("b", [NS], F32, kind="ExternalInput")
    out_d = nc.dram_tensor("out", [t_total, NS], F32, kind="ExternalOutput")

    with tile.TileContext(nc) as tc:
        with (
            tc.tile_pool(name="wtp", bufs=1) as wtpool,
            tc.tile_pool(name="consts", bufs=1) as cpool,
            tc.tile_pool(name="aload", bufs=2) as apool,
            tc.tile_pool(name="anib", bufs=1) as nibpool,
            tc.tile_pool(name="awch", bufs=2) as wchpool,
            tc.tile_pool(name="bx", bufs=2) as bxpool,
            tc.tile_pool(name="bout", bufs=2) as bopool,
            tc.tile_pool(name="ps", bufs=1, space=bass.MemorySpace.PSUM) as pspool,
            tc.tile_pool(name="ptr", bufs=2, space=bass.MemorySpace.PSUM) as trpool,
        ):
            # persistent dequantized W.T: [128 k-low, KC k-chunk, NS n] bf16
            wT = wtpool.tile([128, KC, NS], BF16)

            ident = cpool.tile([128, 128], BF16)
            make_identity(nc, ident[:])
            ones_t = cpool.tile([1, 128], BF16)
            nc.vector.memset(ones_t[:], 1.0)
            bias_row = cpool.tile([1, NS], BF16)
            nc.gpsimd.dma_start(bias_row[:], b_d[:].rearrange("(o n) -> o n", o=1))

            # bias broadcast to all 128 partitions via outer-product matmuls
            bias_full = cpool.tile([128, NS], BF16)
            for nb in range(2):
                bias_ps = pspool.tile([128, MMN], F32, name=f"ps{nb}", bufs=2)
                nc.tensor.matmul(
                    bias_ps[:], ones_t[:],
                    bias_row[:, nb * MMN:(nb + 1) * MMN],
                    start=True, stop=True)
                nc.scalar.copy(bias_full[:, nb * MMN:(nb + 1) * MMN], bias_ps[:])
            bias_psB = pspool.tile([128, 2 * MMN], F32, name="psB", bufs=1)
            for h in range(2):
                nb = 2 + h
                nc.tensor.matmul(
                    bias_psB[:, h * MMN:(h + 1) * MMN], ones_t[:],
                    bias_row[:, nb * MMN:(nb + 1) * MMN],
                    start=True, stop=True)
            nc.scalar.copy(bias_full[:, 2 * MMN:], bias_psB[:])

            # ---- Phase A: dequantize weight slab, n-chunks of 128 rows
            for j in range(NS // 128):
                n0 = j * 128
                qw_t = apool.tile([128, K // PACK], I32)
                nc.sync.dma_start(qw_t[:], qw_d[n0:n0 + 128, :])
                qz_t = apool.tile([128, G // PACK], I32)
                nc.sync.dma_start(qz_t[:], qz_d[n0:n0 + 128, :])
                sc_t = apool.tile([128, G], F32)
                nc.sync.dma_start(sc_t[:], sc_d[n0:n0 + 128, :])

                zi_t = apool.tile([128, G], I32)
                for i in range(PACK):
                    nc.vector.tensor_scalar(
                        out=zi_t[:, i::PACK], in0=qz_t[:],
                        scalar1=4 * i, scalar2=0xF, op0=_LSR, op1=_AND)
                z_t = apool.tile([128, G], F32)
                nc.vector.tensor_copy(z_t[:], zi_t[:])
                # neg_zs = -z*s, the per-group bias for ACT-side dequant
                neg_zs = apool.tile([128, G], F32)
                nc.vector.scalar_tensor_tensor(
                    out=neg_zs[:], in0=z_t[:], scalar=-1.0,
                    in1=sc_t[:], op0=_MUL, op1=_MUL)

                w_t = wchpool.tile([128, K], BF16)
                for h in range(2):
                    nib_t = nibpool.tile([128, K // 2], I32)
                    for i in range(PACK):
                        nc.vector.tensor_scalar(
                            out=nib_t[:, i::PACK],
                            in0=qw_t[:, h * (K // PACK // 2):(h + 1) * (K // PACK // 2)],
                            scalar1=4 * i, scalar2=0xF, op0=_LSR, op1=_AND)
                    # per-group (q-z)*s: split between DVE (tensor_scalar) and
                    # ACT (activation: Identity(nib*s + (-z*s)))
                    for g16 in range(G // 2):
                        g = h * (G // 2) + g16
                        if g16 < 8:
                            nc.vector.tensor_scalar(
                                out=w_t[:, g * GS:(g + 1) * GS],
                                in0=nib_t[:, g16 * GS:(g16 + 1) * GS],
                                scalar1=z_t[:, g:g + 1], scalar2=sc_t[:, g:g + 1],
                                op0=_SUB, op1=_MUL)
                        else:
                            nc.scalar.activation(
                                out=w_t[:, g * GS:(g + 1) * GS],
                                in_=nib_t[:, g16 * GS:(g16 + 1) * GS],
                                func=mybir.ActivationFunctionType.Identity,
                                bias=neg_zs[:, g:g + 1], scale=sc_t[:, g:g + 1])

                # transpose 128x128 tiles on PE (otherwise idle), 8 per PSUM
                # scratch bank, batched ACT copy into the resident wT
                for q in range(KC // 8):
                    tr_t = trpool.tile([128, 1024], BF16)
                    for r in range(8):
                        c = q * 8 + r
                        nc.tensor.transpose(
                            tr_t[:, r * 128:(r + 1) * 128],
                            w_t[:, c * 128:(c + 1) * 128], ident[:])
                    src = tr_t[:].rearrange("p (c n) -> p c n", c=8)
                    nc.scalar.copy(wT[:, q * 8:(q + 1) * 8, n0:n0 + 128], src)

            # ---- Phase B: stream tokens
            for ti in range(nt):
                xT_t = bxpool.tile([128, K], BF16)
                nc.sync.dma_start(xT_t[:], xt_d[ti])
                o_t = bopool.tile([128, NS], F32)
                # nb0/nb1: per-group 1-bank tiles, bufs=2, drained immediately
                # -> during phase A several chunks' nb0 groups can run as soon
                # as the first weight quad lands, instead of serializing on an
                # undrained pair tile
                for nb in range(2):
                    sl = slice(nb * MMN, (nb + 1) * MMN)
                    ps = pspool.tile([128, MMN], F32, name=f"ps{nb}", bufs=2)
                    for c in range(KC):
                        nc.tensor.matmul(
                            ps[:], xT_t[:, c * 128:(c + 1) * 128],
                            wT[:, c, sl],
                            start=(c == 0), stop=(c == KC - 1))
                    nc.vector.scalar_tensor_tensor(
                        out=o_t[:, sl], in0=ps[:], scalar=0.0,
                        in1=bias_full[:, sl], op0=_BYP, op1=_ADD)
                # nb2/nb3: shared 2-bank pair tile, one drain
                psB = pspool.tile([128, 2 * MMN], F32, name="psB", bufs=1)
                for h in range(2):
                    nb = 2 + h
                    sl = slice(nb * MMN, (nb + 1) * MMN)
                    for c in range(KC):
                        nc.tensor.matmul(
                            psB[:, h * MMN:(h + 1) * MMN],
                            xT_t[:, c * 128:(c + 1) * 128],
                            wT[:, c, sl],
                            start=(c == 0), stop=(c == KC - 1))
                nc.vector.scalar_tensor_tensor(
                    out=o_t[:, 2 * MMN:], in0=psB[:], scalar=0.0,
                    in1=bias_full[:, 2 * MMN:], op0=_BYP, op1=_ADD)
                nc.scalar.dma_start(out_d[ti * TCH:(ti + 1) * TCH, :], o_t[:])

    nc.compile()
    return nc


_nc_cache = {}


def _get_nc(t_total: int = T):
    if t_total not in _nc_cache:
        _nc_cache[t_total] = build(t_total)
    return _nc_cache[t_total]


def _prep_x(x, t_total: int = T):
    """f32 [B,S,K] -> bf16 [nt, 128 k-low, (KC k-chunk, 128 tok)] pre-transposed."""
    nt = t_total // TCH
    xb = np.asarray(x).reshape(t_total, K).astype(ml_dtypes.bfloat16)
    # [ti, t, c, kl] -> [ti, kl, c, t]
    xr = xb.reshape(nt, TCH, KC, 128).transpose(0, 3, 2, 1)
    return np.ascontiguousarray(xr).reshape(nt, 128, K)


def kernel(x, qweight, qzeros, scales, bias, trace=False):
    xt = _prep_x(x)
    in_maps = []
    for c in range(NCORES):
        sl = slice(c * NS, (c + 1) * NS)
        in_maps.append({
            "xt": xt,
            "qw": np.ascontiguousarray(qweight[sl]),
            "qz": np.ascontiguousarray(qzeros[sl]),
            "sc": np.ascontiguousarray(scales[sl]),
            "b": np.ascontiguousarray(bias[sl]),
        })
    nc = _get_nc()
    res = run_bass_kernel_spmd(nc, in_maps, core_ids=list(range(NCORES)),
                               trace=trace)
    out = np.concatenate([r["out"] for r in res.results], axis=1)
    out = out.reshape(B, S, N).astype(np.float32, copy=False)
    if trace:
        return out, res
    return out
